# revision 6
# baseline (speedup 1.0000x reference)
"""Trainium2 Bass kernel for CustomBSplineLayer.

Computes out[b,o] = sum_{i,g} spline(x)[b,i,g] * coef[o,i,g] where
spline is an order-3 (cubic) B-spline basis on uniform knots applied to
tanh(x).

Math (validated against the reference recursion to 4e-7):
  u = 3.5*tanh(x) + 3.5           in (0, 7)
  a_g = |u - 2 - g|               g = 0..6 (plane 7 is identically 0)
  p_g = relu(2 - a_g), q_g = relu(1 - a_g)
  basis_g = (p_g^3 - 4 q_g^3) / 6     (1/6 folded into coef on host)

Per-core layout (data-parallel over batch, 8 cores x 512 rows):
  - host pre-transposes x to [i, b] and coef to [i, g*o] bf16 (scaled
    by 1/6), so each i-tile's coef arrives as ONE 14KB/partition DMA
    (the first i-tile's is split per-plane so plane 0 lands early).
  - engine split per i-tile (each engine's work stays under the PE's
    ~12us/i-tile of matmul streaming):
      ACT : tanh, 7x Abs(3.5t + (1.5-g)), wide Square -> p^2
      DVE : two wide dual-op tensor_scalar clamps (4x packed mode),
            two wide cubes (tensor_tensor, 2x mode)
      Pool: 4*q^2 via scalar_tensor_tensor (folds the 4) and the final
            combine spl = q3w - p3w
    Sign trick: pw = min(a-2, 0) = -p so pw^3 = -p^3; q3w = -4q^3;
    then q3w - p3w = p^3 - 4q^3 exactly.
  - chains are emitted in 2 chunks (7 for the first i-tile) so the
    5-stage dependency chain pipelines against the PE instead of
    stalling it.
  - matmul runs in bf16 (1 col/cycle), accumulating all 56 k-tiles
    into 8 PSUM banks [128b x 512o]; the drain spreads PSUM->SBUF
    copies across ACT/DVE/Pool.
"""

import sys

sys.path.insert(0, "/opt/trn_rl_repo")

import numpy as np
from contextlib import ExitStack

import concourse.bass as bass
import concourse.tile as tile
from concourse import bacc, mybir
from concourse.bass_utils import run_bass_kernel_spmd

F32 = mybir.dt.float32
BF16 = mybir.dt.bfloat16
AF = mybir.ActivationFunctionType
OP = mybir.AluOpType

B, I, O = 4096, 1024, 1024
G = 7                    # active basis planes (plane 7 == 0)
NCORES = 8
BC = B // NCORES         # 512 batch rows per core
IT = I // 128            # 8 i-tiles
KT = IT * G              # 56 k-tiles of 128
WID = G * BC             # 3584: wide free-dim (7 planes x 512 b)

MM_DT = BF16

LAST_RESULT = None  # BassKernelResults of the most recent run (for test.py)

_cache = {}


def _to_bf16(a: np.ndarray) -> np.ndarray:
    """Round fp32 -> bf16 (round-to-nearest-even), as ml_dtypes array."""
    dt = mybir.dt.np(BF16)
    return np.ascontiguousarray(a, dtype=np.float32).astype(dt)


def _build_nc(repeats: int = 1):
    nc = bacc.Bacc("TRN2", target_bir_lowering=False, debug=False)
    xT = nc.dram_tensor("xT", [I, BC], F32, kind="ExternalInput").ap()
    coefT = nc.dram_tensor("coefT", [I, G * O], MM_DT, kind="ExternalInput").ap()
    y = nc.dram_tensor("y", [BC, O], BF16, kind="ExternalOutput").ap()

    with tile.TileContext(nc) as tc, ExitStack() as ctx:
        xt_pool = ctx.enter_context(tc.tile_pool(name="xt", bufs=2))
        small = ctx.enter_context(tc.tile_pool(name="small", bufs=2))
        wide = ctx.enter_context(tc.tile_pool(name="wide", bufs=2))
        spl_pool = ctx.enter_context(tc.tile_pool(name="spl", bufs=2))
        rhs_pool = ctx.enter_context(tc.tile_pool(name="rhs", bufs=2))
        out_pool = ctx.enter_context(tc.tile_pool(name="ot", bufs=1))
        psum_pool = ctx.enter_context(
            tc.tile_pool(name="psum", bufs=1, space=bass.MemorySpace.PSUM)
        )

        consts = ctx.enter_context(tc.tile_pool(name="consts", bufs=1))
        bias_a = []
        for g in range(G):
            bt = consts.tile([128, 1], F32, tag=f"bias_a{g}", name=f"bias_a{g}")
            nc.gpsimd.memset(bt[:], float(1.5 - g))
            bias_a.append(bt)

        # 8 PSUM banks: [m-tile 0..3] x [o-half 0..1], each [128, 512] f32
        psum = [
            [
                psum_pool.tile(
                    [128, 512], F32, tag=f"ps{m}_{h}", name=f"ps{m}_{h}"
                )
                for h in range(2)
            ]
            for m in range(4)
        ]

        def emit_front(rep, it, split_coef=False):
            """coef DMA + x DMA + tanh + per-plane |3.5t + (1.5-g)| (ACT)."""
            xt = xt_pool.tile([128, BC], F32, tag="xt", name=f"xt{rep}_{it}")
            nc.sync.dma_start(xt[:], xT[it * 128 : (it + 1) * 128, :])
            rhs = rhs_pool.tile([128, G * O], MM_DT, tag="rhs", name=f"rhs{rep}_{it}")
            row = slice(it * 128, (it + 1) * 128)
            if split_coef:
                for g in range(G):
                    nc.sync.dma_start(
                        rhs[:, g * O : (g + 1) * O], coefT[row, g * O : (g + 1) * O]
                    )
            else:
                nc.sync.dma_start(rhs[:], coefT[row, :])
            t = small.tile([128, BC], BF16, tag="t", name=f"t{rep}_{it}")
            nc.scalar.activation(t[:], xt[:], AF.Tanh)
            aw = wide.tile([128, WID], BF16, tag="a", name=f"aw{rep}_{it}")
            for g in range(G):
                nc.scalar.activation(
                    aw[:, g * BC : (g + 1) * BC],
                    t[:],
                    AF.Abs,
                    bias=bias_a[g][:],
                    scale=3.5,
                )
            return aw, rhs

        def emit_back(rep, it, aw, rhs, kt, chunks=2):
            """Clamps + cubes + combine, then this i-tile's 56 matmuls.

            pw  = min(a-2,0) = -p        (DVE tensor_scalar, 4x mode)
            qw  = min(a-1,0) = -q        (DVE)
            p2w = Square(pw) = p^2       (ACT)
            q2w = (qw*4)*qw  = 4q^2      (Pool scalar_tensor_tensor)
            p3w = p2w*pw     = -p^3      (DVE)
            q3w = q2w*qw     = -4q^3     (DVE)
            spl = q3w - p3w  = p^3-4q^3  (Pool)
            """
            pw = wide.tile([128, WID], BF16, tag="p", name=f"pw{rep}_{it}")
            qw = wide.tile([128, WID], BF16, tag="q", name=f"qw{rep}_{it}")
            p2 = wide.tile([128, WID], BF16, tag="p2", name=f"p2{rep}_{it}")
            q2 = wide.tile([128, WID], BF16, tag="q2", name=f"q2{rep}_{it}")
            spl = spl_pool.tile([128, WID], MM_DT, tag="spl", name=f"spl{rep}_{it}")
            cw = WID // chunks
            for c in range(chunks):
                s = slice(c * cw, (c + 1) * cw)
                nc.vector.tensor_scalar(pw[:, s], aw[:, s], 2.0, 0.0, OP.subtract, OP.min)
                nc.vector.tensor_scalar(qw[:, s], aw[:, s], 1.0, 0.0, OP.subtract, OP.min)
                nc.scalar.activation(p2[:, s], pw[:, s], AF.Square)
                nc.gpsimd.scalar_tensor_tensor(
                    q2[:, s], qw[:, s], 4.0, qw[:, s], OP.mult, OP.mult
                )
                nc.vector.tensor_tensor(p2[:, s], p2[:, s], pw[:, s], OP.mult)
                nc.vector.tensor_tensor(q2[:, s], q2[:, s], qw[:, s], OP.mult)
                nc.gpsimd.tensor_tensor(spl[:, s], q2[:, s], p2[:, s], OP.subtract)
                # matmuls for the planes completed by this chunk
                g_lo = (c * cw) // BC
                g_hi = ((c + 1) * cw) // BC
                for g in range(g_lo, g_hi):
                    first = kt == 0
                    last = kt == KT - 1
                    for m in range(4):
                        lhsT = spl[:, g * BC + m * 128 : g * BC + (m + 1) * 128]
                        for h in range(2):
                            nc.tensor.matmul(
                                psum[m][h][:],
                                lhsT,
                                rhs[:, g * O + h * 512 : g * O + (h + 1) * 512],
                                start=first,
                                stop=last,
                            )
                    kt += 1
            return kt

        def emit_back0(rep, it, aw, rhs, kt):
            """Startup variant: fully per-plane interleaved chain so the PE
            starts within a few us. ACT: p2, q2(=4q^2 via Square scale=2);
            DVE: pw, qw, p3; Pool: q3, spl."""
            pw = wide.tile([128, WID], BF16, tag="p", name=f"pw{rep}_{it}")
            qw = wide.tile([128, WID], BF16, tag="q", name=f"qw{rep}_{it}")
            p2 = wide.tile([128, WID], BF16, tag="p2", name=f"p2{rep}_{it}")
            q2 = wide.tile([128, WID], BF16, tag="q2", name=f"q2{rep}_{it}")
            spl = spl_pool.tile([128, WID], MM_DT, tag="spl", name=f"spl{rep}_{it}")
            for g in range(G):
                s = slice(g * BC, (g + 1) * BC)
                nc.vector.tensor_scalar(pw[:, s], aw[:, s], 2.0, 0.0, OP.subtract, OP.min)
                nc.vector.tensor_scalar(qw[:, s], aw[:, s], 1.0, 0.0, OP.subtract, OP.min)
                nc.scalar.activation(p2[:, s], pw[:, s], AF.Square)
                nc.scalar.activation(q2[:, s], qw[:, s], AF.Square, scale=2.0)
                nc.vector.tensor_tensor(p2[:, s], p2[:, s], pw[:, s], OP.mult)
                nc.gpsimd.tensor_tensor(q2[:, s], q2[:, s], qw[:, s], OP.mult)
                nc.gpsimd.tensor_tensor(spl[:, s], q2[:, s], p2[:, s], OP.subtract)
                first = kt == 0
                last = kt == KT - 1
                for m in range(4):
                    lhsT = spl[:, g * BC + m * 128 : g * BC + (m + 1) * 128]
                    for h in range(2):
                        nc.tensor.matmul(
                            psum[m][h][:],
                            lhsT,
                            rhs[:, g * O + h * 512 : g * O + (h + 1) * 512],
                            start=first,
                            stop=last,
                        )
                kt += 1
            return kt

        for _rep in range(repeats):
            kt = 0
            first_rep = _rep == 0
            aw, rhs = emit_front(_rep, 0, split_coef=first_rep)
            prev = (aw, rhs)
            for it in range(1, IT):
                aw, rhs = emit_front(_rep, it)
                if it == 1 and first_rep:
                    kt = emit_back0(_rep, 0, *prev, kt)
                else:
                    kt = emit_back(_rep, it - 1, *prev, kt)
                prev = (aw, rhs)
            kt = emit_back(_rep, IT - 1, *prev, kt)

            # drain: PSUM -> SBUF copies spread across engines, then DMA out
            for m in range(4):
                ot = out_pool.tile([128, O], BF16, tag=f"ot{m}", name=f"ot{_rep}_{m}")
                for h in range(2):
                    src = psum[m][h][:]
                    dst = ot[:, h * 512 : (h + 1) * 512]
                    eng = (2 * m + h) % 3
                    if eng == 0:
                        nc.scalar.copy(dst, src)
                    elif eng == 1:
                        nc.vector.tensor_scalar(dst, src, 1.0, None, OP.mult)
                    else:
                        nc.gpsimd.tensor_scalar(dst, src, 1.0, None, OP.mult)
                nc.sync.dma_start(y[m * 128 : (m + 1) * 128, :], ot[:])

    nc.compile()
    return nc


def prep_in_maps(x: np.ndarray, coef: np.ndarray):
    """Host-side prep shared by kernel() and test.py."""
    xT = np.ascontiguousarray(np.asarray(x, dtype=np.float32).T)  # [I, B]
    # coef[o,i,g] -> [i, g, o] with the 1/6 basis normalization folded in
    coefT = _to_bf16(
        (np.asarray(coef, dtype=np.float32).transpose(1, 2, 0)[:, :G, :] / 6.0)
        .reshape(I, G * O)
    )
    return [
        {
            "xT": np.ascontiguousarray(xT[:, c * BC : (c + 1) * BC]),
            "coefT": coefT,
        }
        for c in range(NCORES)
    ]


def kernel(x: np.ndarray, coef: np.ndarray) -> np.ndarray:
    global LAST_RESULT
    x = np.asarray(x, dtype=np.float32)
    coef = np.asarray(coef, dtype=np.float32)
    assert x.shape == (B, I) and coef.shape == (O, I, 8)

    if "nc" not in _cache:
        _cache["nc"] = _build_nc()
    nc = _cache["nc"]

    in_maps = prep_in_maps(x, coef)
    res = run_bass_kernel_spmd(nc, in_maps, list(range(NCORES)))
    LAST_RESULT = res
    out = np.concatenate([res.results[c]["y"] for c in range(NCORES)], axis=0)
    return np.ascontiguousarray(out.astype(np.float32))


if __name__ == "__main__":
    rng = np.random.default_rng(0)
    x = rng.standard_normal((B, I), dtype=np.float32)
    coef = rng.standard_normal((O, I, 8), dtype=np.float32) * 0.1
    out = kernel(x, coef)
    print("out", out.shape, out.dtype, float(np.abs(out).max()))


# revision 7
# speedup vs baseline: 1.0008x; 1.0008x over previous
"""Trainium2 Bass kernel for CustomBSplineLayer.

Computes out[b,o] = sum_{i,g} spline(x)[b,i,g] * coef[o,i,g] where
spline is an order-3 (cubic) B-spline basis on uniform knots applied to
tanh(x).

Math (validated against the reference recursion to 4e-7):
  u = 3.5*tanh(x) + 3.5           in (0, 7)
  a_g = |u - 2 - g|               g = 0..6 (plane 7 is identically 0)
  p_g = relu(2 - a_g), q_g = relu(1 - a_g)
  basis_g = (p_g^3 - 4 q_g^3) / 6     (1/6 folded into coef on host)

Per-core layout (data-parallel over batch, 8 cores x 512 rows):
  - host pre-transposes x to [i, b] and coef to [i, g*o] bf16 (scaled
    by 1/6), so each i-tile's coef arrives as ONE 14KB/partition DMA
    (the first i-tile's is split per-plane so plane 0 lands early).
  - engine split per i-tile (each engine's work stays under the PE's
    ~12us/i-tile of matmul streaming):
      ACT : tanh, 7x Abs(3.5t + (1.5-g)), wide Square -> p^2
      DVE : two wide dual-op tensor_scalar clamps (4x packed mode),
            two wide cubes (tensor_tensor, 2x mode)
      Pool: 4*q^2 via scalar_tensor_tensor (folds the 4) and the final
            combine spl = q3w - p3w
    Sign trick: pw = min(a-2, 0) = -p so pw^3 = -p^3; q3w = -4q^3;
    then q3w - p3w = p^3 - 4q^3 exactly.
  - chains are emitted in 2 chunks (7 for the first i-tile) so the
    5-stage dependency chain pipelines against the PE instead of
    stalling it.
  - matmul runs in bf16 (1 col/cycle), accumulating all 56 k-tiles
    into 8 PSUM banks [128b x 512o]; the drain spreads PSUM->SBUF
    copies across ACT/DVE/Pool.
"""

import sys

sys.path.insert(0, "/opt/trn_rl_repo")

import numpy as np
from contextlib import ExitStack

import concourse.bass as bass
import concourse.tile as tile
from concourse import bacc, mybir
from concourse.bass_utils import run_bass_kernel_spmd

F32 = mybir.dt.float32
BF16 = mybir.dt.bfloat16
AF = mybir.ActivationFunctionType
OP = mybir.AluOpType

B, I, O = 4096, 1024, 1024
G = 7                    # active basis planes (plane 7 == 0)
NCORES = 8
BC = B // NCORES         # 512 batch rows per core
IT = I // 128            # 8 i-tiles
KT = IT * G              # 56 k-tiles of 128
WID = G * BC             # 3584: wide free-dim (7 planes x 512 b)

MM_DT = BF16

LAST_RESULT = None  # BassKernelResults of the most recent run (for test.py)

_cache = {}


def _to_bf16(a: np.ndarray) -> np.ndarray:
    """Round fp32 -> bf16 (round-to-nearest-even), as ml_dtypes array."""
    dt = mybir.dt.np(BF16)
    return np.ascontiguousarray(a, dtype=np.float32).astype(dt)


def _build_nc(repeats: int = 1):
    nc = bacc.Bacc("TRN2", target_bir_lowering=False, debug=False)
    xT = nc.dram_tensor("xT", [I, BC], F32, kind="ExternalInput").ap()
    coefT = nc.dram_tensor("coefT", [I, G * O], MM_DT, kind="ExternalInput").ap()
    y = nc.dram_tensor("y", [BC, O], BF16, kind="ExternalOutput").ap()

    with tile.TileContext(nc) as tc, ExitStack() as ctx:
        xt_pool = ctx.enter_context(tc.tile_pool(name="xt", bufs=2))
        small = ctx.enter_context(tc.tile_pool(name="small", bufs=2))
        wide = ctx.enter_context(tc.tile_pool(name="wide", bufs=2))
        spl_pool = ctx.enter_context(tc.tile_pool(name="spl", bufs=2))
        rhs_pool = ctx.enter_context(tc.tile_pool(name="rhs", bufs=2))
        out_pool = ctx.enter_context(tc.tile_pool(name="ot", bufs=1))
        psum_pool = ctx.enter_context(
            tc.tile_pool(name="psum", bufs=1, space=bass.MemorySpace.PSUM)
        )

        consts = ctx.enter_context(tc.tile_pool(name="consts", bufs=1))
        bias_a = []
        for g in range(G):
            bt = consts.tile([128, 1], F32, tag=f"bias_a{g}", name=f"bias_a{g}")
            nc.gpsimd.memset(bt[:], float(1.5 - g))
            bias_a.append(bt)

        # 8 PSUM banks: [m-tile 0..3] x [o-half 0..1], each [128, 512] f32
        psum = [
            [
                psum_pool.tile(
                    [128, 512], F32, tag=f"ps{m}_{h}", name=f"ps{m}_{h}"
                )
                for h in range(2)
            ]
            for m in range(4)
        ]

        def emit_front(rep, it, split_coef=False):
            """coef DMA + x DMA + tanh + per-plane |3.5t + (1.5-g)| (ACT)."""
            xt = xt_pool.tile([128, BC], F32, tag="xt", name=f"xt{rep}_{it}")
            nc.sync.dma_start(xt[:], xT[it * 128 : (it + 1) * 128, :])
            rhs = rhs_pool.tile([128, G * O], MM_DT, tag="rhs", name=f"rhs{rep}_{it}")
            row = slice(it * 128, (it + 1) * 128)
            if split_coef:
                for g in range(G):
                    nc.sync.dma_start(
                        rhs[:, g * O : (g + 1) * O], coefT[row, g * O : (g + 1) * O]
                    )
            else:
                nc.sync.dma_start(rhs[:], coefT[row, :])
            t = small.tile([128, BC], BF16, tag="t", name=f"t{rep}_{it}")
            nc.scalar.activation(t[:], xt[:], AF.Tanh)
            aw = wide.tile([128, WID], BF16, tag="a", name=f"aw{rep}_{it}")
            for g in range(G):
                nc.scalar.activation(
                    aw[:, g * BC : (g + 1) * BC],
                    t[:],
                    AF.Abs,
                    bias=bias_a[g][:],
                    scale=3.5,
                )
            return aw, rhs

        def emit_back(rep, it, aw, rhs, kt, chunks=2):
            """Clamps + cubes + combine, then this i-tile's 56 matmuls.

            pw  = min(a-2,0) = -p        (DVE tensor_scalar, 4x mode)
            qw  = min(a-1,0) = -q        (DVE)
            p2w = Square(pw) = p^2       (ACT)
            q2w = (qw*4)*qw  = 4q^2      (Pool scalar_tensor_tensor)
            p3w = p2w*pw     = -p^3      (DVE)
            q3w = q2w*qw     = -4q^3     (DVE)
            spl = q3w - p3w  = p^3-4q^3  (Pool)
            """
            pw = wide.tile([128, WID], BF16, tag="p", name=f"pw{rep}_{it}")
            qw = wide.tile([128, WID], BF16, tag="q", name=f"qw{rep}_{it}")
            p2 = wide.tile([128, WID], BF16, tag="p2", name=f"p2{rep}_{it}")
            q2 = wide.tile([128, WID], BF16, tag="q2", name=f"q2{rep}_{it}")
            spl = spl_pool.tile([128, WID], MM_DT, tag="spl", name=f"spl{rep}_{it}")
            cw = WID // chunks
            for c in range(chunks):
                s = slice(c * cw, (c + 1) * cw)
                nc.vector.tensor_scalar(pw[:, s], aw[:, s], 2.0, 0.0, OP.subtract, OP.min)
                nc.vector.tensor_scalar(qw[:, s], aw[:, s], 1.0, 0.0, OP.subtract, OP.min)
                nc.scalar.activation(p2[:, s], pw[:, s], AF.Square)
                nc.gpsimd.scalar_tensor_tensor(
                    q2[:, s], qw[:, s], 4.0, qw[:, s], OP.mult, OP.mult
                )
                nc.vector.tensor_tensor(p2[:, s], p2[:, s], pw[:, s], OP.mult)
                nc.vector.tensor_tensor(q2[:, s], q2[:, s], qw[:, s], OP.mult)
                nc.gpsimd.tensor_tensor(spl[:, s], q2[:, s], p2[:, s], OP.subtract)
                # matmuls for the planes completed by this chunk
                g_lo = (c * cw) // BC
                g_hi = ((c + 1) * cw) // BC
                for g in range(g_lo, g_hi):
                    first = kt == 0
                    last = kt == KT - 1
                    for m in range(4):
                        lhsT = spl[:, g * BC + m * 128 : g * BC + (m + 1) * 128]
                        for h in range(2):
                            nc.tensor.matmul(
                                psum[m][h][:],
                                lhsT,
                                rhs[:, g * O + h * 512 : g * O + (h + 1) * 512],
                                start=first,
                                stop=last,
                            )
                    kt += 1
            return kt

        def emit_back0(rep, it, aw, rhs, kt):
            """Startup variant: fully per-plane interleaved chain so the PE
            starts within a few us. ACT: p2, q2(=4q^2 via Square scale=2);
            DVE: pw, qw, p3; Pool: q3, spl."""
            pw = wide.tile([128, WID], BF16, tag="p", name=f"pw{rep}_{it}")
            qw = wide.tile([128, WID], BF16, tag="q", name=f"qw{rep}_{it}")
            p2 = wide.tile([128, WID], BF16, tag="p2", name=f"p2{rep}_{it}")
            q2 = wide.tile([128, WID], BF16, tag="q2", name=f"q2{rep}_{it}")
            spl = spl_pool.tile([128, WID], MM_DT, tag="spl", name=f"spl{rep}_{it}")
            for g in range(G):
                s = slice(g * BC, (g + 1) * BC)
                nc.vector.tensor_scalar(pw[:, s], aw[:, s], 2.0, 0.0, OP.subtract, OP.min)
                nc.vector.tensor_scalar(qw[:, s], aw[:, s], 1.0, 0.0, OP.subtract, OP.min)
                nc.scalar.activation(p2[:, s], pw[:, s], AF.Square)
                nc.scalar.activation(q2[:, s], qw[:, s], AF.Square, scale=2.0)
                nc.vector.tensor_tensor(p2[:, s], p2[:, s], pw[:, s], OP.mult)
                nc.gpsimd.tensor_tensor(q2[:, s], q2[:, s], qw[:, s], OP.mult)
                nc.gpsimd.tensor_tensor(spl[:, s], q2[:, s], p2[:, s], OP.subtract)
                first = kt == 0
                last = kt == KT - 1
                for m in range(4):
                    lhsT = spl[:, g * BC + m * 128 : g * BC + (m + 1) * 128]
                    for h in range(2):
                        nc.tensor.matmul(
                            psum[m][h][:],
                            lhsT,
                            rhs[:, g * O + h * 512 : g * O + (h + 1) * 512],
                            start=first,
                            stop=last,
                        )
                kt += 1
            return kt

        for _rep in range(repeats):
            kt = 0
            first_rep = _rep == 0
            aw, rhs = emit_front(_rep, 0, split_coef=first_rep)
            if first_rep:
                # tile-0 chain emitted before front(1) so its ACT ops are
                # not stuck behind tile 1's tanh/abs in the ACT queue
                kt = emit_back0(_rep, 0, aw, rhs, kt)
                prev = None
            else:
                prev = (aw, rhs)
            for it in range(1, IT):
                aw, rhs = emit_front(_rep, it)
                if prev is not None:
                    kt = emit_back(_rep, it - 1, *prev, kt)
                prev = (aw, rhs)
            kt = emit_back(_rep, IT - 1, *prev, kt)

            # drain: PSUM -> SBUF copies spread across engines, then DMA out
            for m in range(4):
                ot = out_pool.tile([128, O], BF16, tag=f"ot{m}", name=f"ot{_rep}_{m}")
                for h in range(2):
                    src = psum[m][h][:]
                    dst = ot[:, h * 512 : (h + 1) * 512]
                    eng = (2 * m + h) % 3
                    if eng == 0:
                        nc.scalar.copy(dst, src)
                    elif eng == 1:
                        nc.vector.tensor_scalar(dst, src, 1.0, None, OP.mult)
                    else:
                        nc.gpsimd.tensor_scalar(dst, src, 1.0, None, OP.mult)
                nc.sync.dma_start(y[m * 128 : (m + 1) * 128, :], ot[:])

    nc.compile()
    return nc


def prep_in_maps(x: np.ndarray, coef: np.ndarray):
    """Host-side prep shared by kernel() and test.py."""
    xT = np.ascontiguousarray(np.asarray(x, dtype=np.float32).T)  # [I, B]
    # coef[o,i,g] -> [i, g, o] with the 1/6 basis normalization folded in
    coefT = _to_bf16(
        (np.asarray(coef, dtype=np.float32).transpose(1, 2, 0)[:, :G, :] / 6.0)
        .reshape(I, G * O)
    )
    return [
        {
            "xT": np.ascontiguousarray(xT[:, c * BC : (c + 1) * BC]),
            "coefT": coefT,
        }
        for c in range(NCORES)
    ]


def kernel(x: np.ndarray, coef: np.ndarray) -> np.ndarray:
    global LAST_RESULT
    x = np.asarray(x, dtype=np.float32)
    coef = np.asarray(coef, dtype=np.float32)
    assert x.shape == (B, I) and coef.shape == (O, I, 8)

    if "nc" not in _cache:
        _cache["nc"] = _build_nc()
    nc = _cache["nc"]

    in_maps = prep_in_maps(x, coef)
    res = run_bass_kernel_spmd(nc, in_maps, list(range(NCORES)))
    LAST_RESULT = res
    out = np.concatenate([res.results[c]["y"] for c in range(NCORES)], axis=0)
    return np.ascontiguousarray(out.astype(np.float32))


if __name__ == "__main__":
    rng = np.random.default_rng(0)
    x = rng.standard_normal((B, I), dtype=np.float32)
    coef = rng.standard_normal((O, I, 8), dtype=np.float32) * 0.1
    out = kernel(x, coef)
    print("out", out.shape, out.dtype, float(np.abs(out).max()))


# revision 8
# speedup vs baseline: 1.0942x; 1.0933x over previous
"""Trainium2 Bass kernel for CustomBSplineLayer.

Computes out[b,o] = sum_{i,g} spline(x)[b,i,g] * coef[o,i,g] where
spline is an order-3 (cubic) B-spline basis on uniform knots applied to
tanh(x).

Math (validated against the reference recursion to 4e-7):
  u = 3.5*tanh(x) + 3.5           in (0, 7)
  a_g = |u - 2 - g|               g = 0..6 (plane 7 is identically 0)
  p_g = relu(2 - a_g), q_g = relu(1 - a_g)
  basis_g = (p_g^3 - 4 q_g^3) / 6

On-chip everything is computed on the t = tanh(x) scale (a' = a/3.5),
with the 3.5^3 make-up factor and the 1/6 folded into coef on the host
(coef' = coef * 3.5 / 6):
  aw_g = |t + (1.5-g)/3.5|        = a/3.5   (DVE tensor_scalar, 4x mode)
  pw   = min(aw - 2/3.5, 0)       = -p/3.5  (DVE)
  qw   = min(aw - 1/3.5, 0)       = -q/3.5  (DVE)
  p2   = Square(3.5*pw)           = p^2     (ACT, scale folds the 3.5)
  q2   = (qw*49)*qw               = 4q^2    (Pool scalar_tensor_tensor)
  p3   = p2*pw                    = -p^3/3.5
  q3   = q2*qw                    = -4q^3/3.5   (DVE)
  spl  = q3 - p3                  = (p^3-4q^3)/3.5  (Pool)

Per-core layout (data-parallel over batch, 8 cores x 512 rows):
  - host pre-transposes x to [i, b] and coef to [i, g*o] bf16, so each
    i-tile's coef arrives as ONE 14KB/partition DMA (the first i-tile's
    is split per-plane so plane 0 lands early).
  - every engine's per-i-tile work is well under the PE's ~12us of
    matmul streaming; chains are emitted in 2 chunks (per-plane for the
    first i-tile) so the 5-stage dependency chain pipelines against the
    PE instead of stalling it.
  - matmul runs in bf16 (1 col/cycle), accumulating all 56 k-tiles
    into 8 PSUM banks [128b x 512o]; the drain spreads PSUM->SBUF
    copies across ACT/DVE/Pool and outputs bf16 (halves the final DMA).
"""

import sys

sys.path.insert(0, "/opt/trn_rl_repo")

import numpy as np
from contextlib import ExitStack

import concourse.bass as bass
import concourse.tile as tile
from concourse import bacc, mybir
from concourse.bass_utils import run_bass_kernel_spmd

F32 = mybir.dt.float32
BF16 = mybir.dt.bfloat16
AF = mybir.ActivationFunctionType
OP = mybir.AluOpType

B, I, O = 4096, 1024, 1024
G = 7                    # active basis planes (plane 7 == 0)
NCORES = 8
BC = B // NCORES         # 512 batch rows per core
IT = I // 128            # 8 i-tiles
KT = IT * G              # 56 k-tiles of 128
WID = G * BC             # 3584: wide free-dim (7 planes x 512 b)

P_TH = float(2.0 / 3.5)  # clamp thresholds on the tanh scale
Q_TH = float(1.0 / 3.5)

MM_DT = BF16

LAST_RESULT = None  # BassKernelResults of the most recent run (for test.py)

_cache = {}


def _to_bf16(a: np.ndarray) -> np.ndarray:
    """Round fp32 -> bf16 (round-to-nearest-even), as ml_dtypes array."""
    dt = mybir.dt.np(BF16)
    return np.ascontiguousarray(a, dtype=np.float32).astype(dt)


def _build_nc(repeats: int = 1):
    nc = bacc.Bacc("TRN2", target_bir_lowering=False, debug=False)
    xT = nc.dram_tensor("xT", [I, BC], F32, kind="ExternalInput").ap()
    coefT = nc.dram_tensor("coefT", [I, G * O], MM_DT, kind="ExternalInput").ap()
    y = nc.dram_tensor("y", [BC, O], BF16, kind="ExternalOutput").ap()

    with tile.TileContext(nc) as tc, ExitStack() as ctx:
        xt_pool = ctx.enter_context(tc.tile_pool(name="xt", bufs=2))
        small = ctx.enter_context(tc.tile_pool(name="small", bufs=2))
        wide = ctx.enter_context(tc.tile_pool(name="wide", bufs=2))
        spl_pool = ctx.enter_context(tc.tile_pool(name="spl", bufs=2))
        rhs_pool = ctx.enter_context(tc.tile_pool(name="rhs", bufs=2))
        out_pool = ctx.enter_context(tc.tile_pool(name="ot", bufs=1))
        psum_pool = ctx.enter_context(
            tc.tile_pool(name="psum", bufs=1, space=bass.MemorySpace.PSUM)
        )

        # 8 PSUM banks: [m-tile 0..3] x [o-half 0..1], each [128, 512] f32
        psum = [
            [
                psum_pool.tile(
                    [128, 512], F32, tag=f"ps{m}_{h}", name=f"ps{m}_{h}"
                )
                for h in range(2)
            ]
            for m in range(4)
        ]

        def emit_front(rep, it, split_coef=False):
            """coef DMA + x DMA + tanh (ACT) + per-plane abs (DVE)."""
            xt = xt_pool.tile([128, BC], F32, tag="xt", name=f"xt{rep}_{it}")
            nc.sync.dma_start(xt[:], xT[it * 128 : (it + 1) * 128, :])
            rhs = rhs_pool.tile([128, G * O], MM_DT, tag="rhs", name=f"rhs{rep}_{it}")
            row = slice(it * 128, (it + 1) * 128)
            if split_coef:
                for g in range(G):
                    nc.sync.dma_start(
                        rhs[:, g * O : (g + 1) * O], coefT[row, g * O : (g + 1) * O]
                    )
            else:
                nc.sync.dma_start(rhs[:], coefT[row, :])
            t = small.tile([128, BC], BF16, tag="t", name=f"t{rep}_{it}")
            nc.scalar.activation(t[:], xt[:], AF.Tanh)
            aw = wide.tile([128, WID], BF16, tag="a", name=f"aw{rep}_{it}")
            for g in range(G):
                nc.vector.tensor_scalar(
                    aw[:, g * BC : (g + 1) * BC],
                    t[:],
                    float((1.5 - g) / 3.5),
                    0.0,
                    OP.add,
                    OP.abs_max,
                )
            return aw, rhs

        def emit_mm(rep, it, spl, rhs, kt, g_lo, g_hi):
            for g in range(g_lo, g_hi):
                first = kt == 0
                last = kt == KT - 1
                for m in range(4):
                    lhsT = spl[:, g * BC + m * 128 : g * BC + (m + 1) * 128]
                    for h in range(2):
                        nc.tensor.matmul(
                            psum[m][h][:],
                            lhsT,
                            rhs[:, g * O + h * 512 : g * O + (h + 1) * 512],
                            start=first,
                            stop=last,
                        )
                kt += 1
            return kt

        def emit_back(rep, it, aw, rhs, kt, chunks=2):
            """Clamps + cubes + combine (see module docstring), then the
            i-tile's 56 matmuls, per chunk."""
            pw = wide.tile([128, WID], BF16, tag="p", name=f"pw{rep}_{it}")
            qw = wide.tile([128, WID], BF16, tag="q", name=f"qw{rep}_{it}")
            p2 = wide.tile([128, WID], BF16, tag="p2", name=f"p2{rep}_{it}")
            q2 = wide.tile([128, WID], BF16, tag="q2", name=f"q2{rep}_{it}")
            spl = spl_pool.tile([128, WID], MM_DT, tag="spl", name=f"spl{rep}_{it}")
            cw = WID // chunks
            for c in range(chunks):
                s = slice(c * cw, (c + 1) * cw)
                nc.vector.tensor_scalar(pw[:, s], aw[:, s], P_TH, 0.0, OP.subtract, OP.min)
                nc.vector.tensor_scalar(qw[:, s], aw[:, s], Q_TH, 0.0, OP.subtract, OP.min)
                nc.scalar.activation(p2[:, s], pw[:, s], AF.Square, scale=3.5)
                nc.gpsimd.scalar_tensor_tensor(
                    q2[:, s], qw[:, s], 49.0, qw[:, s], OP.mult, OP.mult
                )
                nc.vector.tensor_tensor(p2[:, s], p2[:, s], pw[:, s], OP.mult)
                nc.vector.tensor_tensor(q2[:, s], q2[:, s], qw[:, s], OP.mult)
                nc.gpsimd.tensor_tensor(spl[:, s], q2[:, s], p2[:, s], OP.subtract)
                kt = emit_mm(rep, it, spl, rhs, kt, (c * cw) // BC, ((c + 1) * cw) // BC)
            return kt

        for _rep in range(repeats):
            kt = 0
            first_rep = _rep == 0
            aw, rhs = emit_front(_rep, 0, split_coef=first_rep)
            if first_rep:
                # tile-0 chain per-plane so the PE starts within a few us
                kt = emit_back(_rep, 0, aw, rhs, kt, chunks=G)
                prev = None
            else:
                prev = (aw, rhs)
            for it in range(1, IT):
                aw, rhs = emit_front(_rep, it)
                if prev is not None:
                    kt = emit_back(_rep, it - 1, *prev, kt)
                prev = (aw, rhs)
            kt = emit_back(_rep, IT - 1, *prev, kt)

            # drain: PSUM -> SBUF copies spread across engines, then DMA out
            for m in range(4):
                ot = out_pool.tile([128, O], BF16, tag=f"ot{m}", name=f"ot{_rep}_{m}")
                for h in range(2):
                    src = psum[m][h][:]
                    dst = ot[:, h * 512 : (h + 1) * 512]
                    eng = (2 * m + h) % 3
                    if eng == 0:
                        nc.scalar.copy(dst, src)
                    elif eng == 1:
                        nc.vector.tensor_scalar(dst, src, 1.0, None, OP.mult)
                    else:
                        nc.gpsimd.tensor_scalar(dst, src, 1.0, None, OP.mult)
                nc.sync.dma_start(y[m * 128 : (m + 1) * 128, :], ot[:])

    nc.compile()
    return nc


def prep_in_maps(x: np.ndarray, coef: np.ndarray):
    """Host-side prep shared by kernel() and test.py."""
    xT = np.ascontiguousarray(np.asarray(x, dtype=np.float32).T)  # [I, B]
    # coef[o,i,g] -> [i, g, o]; fold the 1/6 normalization and the 3.5
    # make-up factor for the tanh-scale spline pipeline
    coefT = _to_bf16(
        (np.asarray(coef, dtype=np.float32).transpose(1, 2, 0)[:, :G, :] * (3.5 / 6.0))
        .reshape(I, G * O)
    )
    return [
        {
            "xT": np.ascontiguousarray(xT[:, c * BC : (c + 1) * BC]),
            "coefT": coefT,
        }
        for c in range(NCORES)
    ]


def kernel(x: np.ndarray, coef: np.ndarray) -> np.ndarray:
    global LAST_RESULT
    x = np.asarray(x, dtype=np.float32)
    coef = np.asarray(coef, dtype=np.float32)
    assert x.shape == (B, I) and coef.shape == (O, I, 8)

    if "nc" not in _cache:
        _cache["nc"] = _build_nc()
    nc = _cache["nc"]

    in_maps = prep_in_maps(x, coef)
    res = run_bass_kernel_spmd(nc, in_maps, list(range(NCORES)))
    LAST_RESULT = res
    out = np.concatenate([res.results[c]["y"] for c in range(NCORES)], axis=0)
    return np.ascontiguousarray(out.astype(np.float32))


if __name__ == "__main__":
    rng = np.random.default_rng(0)
    x = rng.standard_normal((B, I), dtype=np.float32)
    coef = rng.standard_normal((O, I, 8), dtype=np.float32) * 0.1
    out = kernel(x, coef)
    print("out", out.shape, out.dtype, float(np.abs(out).max()))


# revision 11
# speedup vs baseline: 1.3356x; 1.2207x over previous
"""Trainium2 Bass kernel for CustomBSplineLayer.

Computes out[b,o] = sum_{i,g} spline(x)[b,i,g] * coef[o,i,g] where
spline is an order-3 (cubic) B-spline basis on uniform knots applied to
tanh(x).

Math (validated against the reference recursion to 4e-7):
  u = 3.5*tanh(x) + 3.5           in (0, 7)
  a_g = |u - 2 - g|               g = 0..6 (plane 7 is identically 0)
  p_g = relu(2 - a_g), q_g = relu(1 - a_g)
  basis_g = (p_g^3 - 4 q_g^3) / 6

On-chip everything is computed on the t = tanh(x) scale (a' = a/3.5),
with the 3.5^3 make-up factor and the 1/6 folded into coef on the host
(coef' = coef * 3.5 / 6):
  aw_g = |t + (1.5-g)/3.5|        = a/3.5   (DVE tensor_scalar, 4x mode)
  pw   = min(aw - 2/3.5, 0)       = -p/3.5  (DVE)
  qw   = min(aw - 1/3.5, 0)       = -q/3.5  (DVE)
  p2   = Square(3.5*pw)           = p^2     (ACT, scale folds the 3.5)
  q2   = (qw*49)*qw               = 4q^2    (Pool scalar_tensor_tensor)
  p3   = p2*pw                    = -p^3/3.5
  q3   = q2*qw                    = -4q^3/3.5   (DVE)
  spl  = q3 - p3                  = (p^3-4q^3)/3.5  (Pool)

Per-core layout (data-parallel over batch, 8 cores x 512 rows):
  - host pre-transposes x to [i, b] and coef to [i, g*o] bf16, so each
    i-tile's coef arrives as ONE 14KB/partition DMA (the first i-tile's
    is split per-plane so plane 0 lands early).
  - every engine's per-i-tile work is well under the PE's ~12us of
    matmul streaming; chains are emitted in 2 chunks (per-plane for the
    first i-tile) so the 5-stage dependency chain pipelines against the
    PE instead of stalling it.
  - matmul runs in bf16 (1 col/cycle), accumulating all 56 k-tiles
    into 8 PSUM banks [128b x 512o]; the drain spreads PSUM->SBUF
    copies across ACT/DVE/Pool and outputs bf16 (halves the final DMA).
"""

import sys

sys.path.insert(0, "/opt/trn_rl_repo")

import numpy as np
from contextlib import ExitStack

import concourse.bass as bass
import concourse.tile as tile
from concourse import bacc, mybir
from concourse.bass_utils import run_bass_kernel_spmd

F32 = mybir.dt.float32
BF16 = mybir.dt.bfloat16
AF = mybir.ActivationFunctionType
OP = mybir.AluOpType

B, I, O = 4096, 1024, 1024
G = 7                    # active basis planes (plane 7 == 0)
NCORES = 8
BC = B // NCORES         # 512 batch rows per core
IT = I // 128            # 8 i-tiles
KT = IT * G              # 56 k-tiles of 128
WID = G * BC             # 3584: wide free-dim (7 planes x 512 b)

P_TH = 2.0  # clamp thresholds (unit grid scale)
Q_TH = 1.0

MM_DT = BF16

LAST_RESULT = None  # BassKernelResults of the most recent run (for test.py)

_cache = {}


def _to_bf16(a: np.ndarray) -> np.ndarray:
    """Round fp32 -> bf16 (round-to-nearest-even), as ml_dtypes array."""
    dt = mybir.dt.np(BF16)
    return np.ascontiguousarray(a, dtype=np.float32).astype(dt)


def _build_nc(repeats: int = 1):
    nc = bacc.Bacc("TRN2", target_bir_lowering=False, debug=False)
    xT = nc.dram_tensor("xT", [I, BC], F32, kind="ExternalInput").ap()
    coefT = nc.dram_tensor("coefT", [I, G * O], MM_DT, kind="ExternalInput").ap()
    y = nc.dram_tensor("y", [BC, O], BF16, kind="ExternalOutput").ap()

    with tile.TileContext(nc) as tc, ExitStack() as ctx:
        xt_pool = ctx.enter_context(tc.tile_pool(name="xt", bufs=2))
        small = ctx.enter_context(tc.tile_pool(name="small", bufs=2))
        wide = ctx.enter_context(tc.tile_pool(name="wide", bufs=2))
        spl_pool = ctx.enter_context(tc.tile_pool(name="spl", bufs=2))
        rhs_pool = ctx.enter_context(tc.tile_pool(name="rhs", bufs=2))
        out_pool = ctx.enter_context(tc.tile_pool(name="ot", bufs=1))
        psum_pool = ctx.enter_context(
            tc.tile_pool(name="psum", bufs=1, space=bass.MemorySpace.PSUM)
        )

        consts = ctx.enter_context(tc.tile_pool(name="consts", bufs=1))
        bias_a = []
        for g in range(G):
            bt = consts.tile([128, 1], F32, tag=f"bias_a{g}", name=f"bias_a{g}")
            nc.gpsimd.memset(bt[:], float(1.5 - g))
            bias_a.append(bt)

        # 8 PSUM banks: [m-tile 0..3] x [o-half 0..1], each [128, 512] f32
        psum = [
            [
                psum_pool.tile(
                    [128, 512], F32, tag=f"ps{m}_{h}", name=f"ps{m}_{h}"
                )
                for h in range(2)
            ]
            for m in range(4)
        ]

        def emit_front(rep, it, split_coef=False):
            """coef DMA + x DMA + tanh (ACT) + per-plane abs (DVE)."""
            xt = xt_pool.tile([128, BC], F32, tag="xt", name=f"xt{rep}_{it}")
            nc.sync.dma_start(xt[:], xT[it * 128 : (it + 1) * 128, :])
            rhs = rhs_pool.tile([128, G * O], MM_DT, tag="rhs", name=f"rhs{rep}_{it}")
            row = slice(it * 128, (it + 1) * 128)
            if split_coef:
                for g in range(G):
                    nc.sync.dma_start(
                        rhs[:, g * O : (g + 1) * O], coefT[row, g * O : (g + 1) * O]
                    )
            else:
                nc.sync.dma_start(rhs[:], coefT[row, :])
            t = small.tile([128, BC], BF16, tag="t", name=f"t{rep}_{it}")
            nc.scalar.activation(t[:], xt[:], AF.Tanh)
            aw = wide.tile([128, WID], BF16, tag="a", name=f"aw{rep}_{it}")
            for g in range(G):
                nc.scalar.activation(
                    aw[:, g * BC : (g + 1) * BC],
                    t[:],
                    AF.Abs,
                    bias=bias_a[g][:],
                    scale=3.5,
                )
            return aw, rhs

        def emit_mm(rep, it, spl, rhs, kt, g_lo, g_hi):
            for g in range(g_lo, g_hi):
                first = kt == 0
                last = kt == KT - 1
                for m in range(4):
                    lhsT = spl[:, g * BC + m * 128 : g * BC + (m + 1) * 128]
                    for h in range(2):
                        nc.tensor.matmul(
                            psum[m][h][:],
                            lhsT,
                            rhs[:, g * O + h * 512 : g * O + (h + 1) * 512],
                            start=first,
                            stop=last,
                        )
                kt += 1
            return kt

        def emit_back(rep, it, aw, rhs, kt, chunks=2):
            """Clamps + cubes + combine (see module docstring), then the
            i-tile's 56 matmuls, per chunk."""
            pw = wide.tile([128, WID], BF16, tag="p", name=f"pw{rep}_{it}")
            qw = wide.tile([128, WID], BF16, tag="q", name=f"qw{rep}_{it}")
            p2 = wide.tile([128, WID], BF16, tag="p2", name=f"p2{rep}_{it}")
            q2 = wide.tile([128, WID], BF16, tag="q2", name=f"q2{rep}_{it}")
            spl = spl_pool.tile([128, WID], MM_DT, tag="spl", name=f"spl{rep}_{it}")
            cw = WID // chunks
            for c in range(chunks):
                s = slice(c * cw, (c + 1) * cw)
                nc.vector.tensor_scalar(pw[:, s], aw[:, s], P_TH, 0.0, OP.subtract, OP.min)
                nc.vector.tensor_scalar(qw[:, s], aw[:, s], Q_TH, 0.0, OP.subtract, OP.min)
                nc.gpsimd.tensor_tensor(p2[:, s], pw[:, s], pw[:, s], OP.mult)
                nc.scalar.activation(q2[:, s], qw[:, s], AF.Square, scale=2.0)
                nc.vector.tensor_tensor(p2[:, s], p2[:, s], pw[:, s], OP.mult)
                nc.vector.tensor_tensor(q2[:, s], q2[:, s], qw[:, s], OP.mult)
                nc.gpsimd.tensor_tensor(spl[:, s], q2[:, s], p2[:, s], OP.subtract)
                kt = emit_mm(rep, it, spl, rhs, kt, (c * cw) // BC, ((c + 1) * cw) // BC)
            return kt

        for _rep in range(repeats):
            kt = 0
            first_rep = _rep == 0
            aw, rhs = emit_front(_rep, 0, split_coef=first_rep)
            if first_rep:
                # tile-0 chain per-plane so the PE starts within a few us
                kt = emit_back(_rep, 0, aw, rhs, kt, chunks=G)
                prev = None
            else:
                prev = (aw, rhs)
            for it in range(1, IT):
                aw, rhs = emit_front(_rep, it)
                if prev is not None:
                    kt = emit_back(_rep, it - 1, *prev, kt)
                prev = (aw, rhs)
            kt = emit_back(_rep, IT - 1, *prev, kt)

            # drain: PSUM -> SBUF copies spread across engines, then DMA out
            for m in range(4):
                ot = out_pool.tile([128, O], BF16, tag=f"ot{m}", name=f"ot{_rep}_{m}")
                for h in range(2):
                    src = psum[m][h][:]
                    dst = ot[:, h * 512 : (h + 1) * 512]
                    # Pool/GPSIMD cannot read PSUM; alternate ACT and DVE
                    if (2 * m + h) % 2 == 0:
                        nc.scalar.copy(dst, src)
                    else:
                        nc.vector.tensor_scalar(dst, src, 1.0, None, OP.mult)
                nc.sync.dma_start(y[m * 128 : (m + 1) * 128, :], ot[:])

    nc.compile()
    return nc


def prep_in_maps(x: np.ndarray, coef: np.ndarray):
    """Host-side prep shared by kernel() and test.py."""
    xT = np.ascontiguousarray(np.asarray(x, dtype=np.float32).T)  # [I, B]
    # coef[o,i,g] -> [i, g, o] with the 1/6 basis normalization folded in
    coefT = _to_bf16(
        (np.asarray(coef, dtype=np.float32).transpose(1, 2, 0)[:, :G, :] / 6.0)
        .reshape(I, G * O)
    )
    return [
        {
            "xT": np.ascontiguousarray(xT[:, c * BC : (c + 1) * BC]),
            "coefT": coefT,
        }
        for c in range(NCORES)
    ]


def kernel(x: np.ndarray, coef: np.ndarray) -> np.ndarray:
    global LAST_RESULT
    x = np.asarray(x, dtype=np.float32)
    coef = np.asarray(coef, dtype=np.float32)
    assert x.shape == (B, I) and coef.shape == (O, I, 8)

    if "nc" not in _cache:
        _cache["nc"] = _build_nc()
    nc = _cache["nc"]

    in_maps = prep_in_maps(x, coef)
    res = run_bass_kernel_spmd(nc, in_maps, list(range(NCORES)))
    LAST_RESULT = res
    out = np.concatenate([res.results[c]["y"] for c in range(NCORES)], axis=0)
    return np.ascontiguousarray(out.astype(np.float32))


if __name__ == "__main__":
    rng = np.random.default_rng(0)
    x = rng.standard_normal((B, I), dtype=np.float32)
    coef = rng.standard_normal((O, I, 8), dtype=np.float32) * 0.1
    out = kernel(x, coef)
    print("out", out.shape, out.dtype, float(np.abs(out).max()))
